# revision 45
# baseline (speedup 1.0000x reference)
"""MoE MLP (E=32 experts, top-2, D=H=1024) on 8 Trainium2 NeuronCores.

Strategy (expert parallel, per sharding hint):
  * Host computes the (tiny) gate: softmax(x @ Wg), top-2, renormalized
    weights, and dispatches tokens per expert into capacity-padded blocks,
    transposed to [D, tokens] (features on SBUF partitions, tokens on the
    matmul moving/free dimension). This is the sharding/all-to-all step.
  * Each of the 8 cores owns 4 experts (W1/W2/b1/b2 shards) and computes
    GELU(x W1 + b1) W2 + b2 for its experts' token blocks.
  * Host combines with the top-2 gate weights (scatter-add).

v2 changes vs the 90us baseline:
  * Variable per-slot capacities: experts are sorted by token count and
    grouped into 4 "slots" of 8; slot capacity = max count in the group
    (rounded to 16). All cores run the same program (SPMD) but total
    padded columns drop from 4*304=1216 to ~1088, cutting the matmul
    stream time proportionally (the kernel is PE-bound at ~0.42ns/col).
  * y written back in bf16 (halves writeback traffic; combine is f32 on
    host).
  * Warmup right-sized: dense 128-col dummy matmuls bridge the NEFF
    preamble to the first real matmul with no PE gap, so the HAM clock
    un-throttles (1.2 -> 2.4 GHz) as early as possible.
  * First slot's x and W1 issued first in small chunks for the earliest
    possible compute start.
"""

import os
import sys
import numpy as np

for _p in ("/root/.axon_site/_ro/trn_rl_repo", "/opt/trn_rl_repo"):
    if _p not in sys.path and os.path.isdir(_p):
        sys.path.append(_p)

E, D, H = 32, 1024, 1024
TOP_K = 2
N_CORES = 8
EPC = E // N_CORES  # experts (slots) per core
ND = D // 128       # d 128-tiles
NH = H // 128       # h 128-tiles

# weight dtype, activation dtype (must both be 16-bit or both 32-bit)
DT_W = os.environ.get("MOE_DT_W", "bfloat16")
DT_W2 = os.environ.get("MOE_DT_W2", "float8e3")   # E3M4 fp8 for W2
DT_A = os.environ.get("MOE_DT_A", "bfloat16")
DT_Y = os.environ.get("MOE_DT_Y", "bfloat16")
N_WARMUP_MM = int(os.environ.get("MOE_WARMUP", "28"))
WARMUP_COLS = int(os.environ.get("MOE_WARMUP_COLS", "256"))
HWDGE_ONLY = os.environ.get("MOE_HWDGE", "0") == "1"

LAST_EXEC_TIME_NS = None

_NC_CACHE = {}


def _chunks_for(cap):
    """Split a slot capacity into matmul chunks of <=512 columns."""
    n = -(-cap // 512)
    w = -(-cap // (n * 16)) * 16
    out = []
    rem = cap
    for _ in range(n):
        c = min(w, rem)
        out.append(c)
        rem -= c
    return tuple(out)


def _build_nc(caps, dt_w_name, dt_w2_name, dt_a_name, dt_y_name,
              hwdge=HWDGE_ONLY):
    import concourse.bass as bass  # noqa: F401
    import concourse.tile as tile
    from concourse import bacc, mybir
    from contextlib import ExitStack

    f32 = mybir.dt.float32
    dt_w = getattr(mybir.dt, dt_w_name)
    dt_w2 = getattr(mybir.dt, dt_w2_name)
    dt_a = getattr(mybir.dt, dt_a_name)
    dt_y = getattr(mybir.dt, dt_y_name)
    S = sum(caps)
    offs = np.concatenate([[0], np.cumsum(caps)]).astype(int)

    nc = bacc.Bacc(
        "TRN2",
        target_bir_lowering=False,
        debug=False,
        enable_asserts=False,
        num_devices=N_CORES,
    )
    xT = nc.dram_tensor("xT", [D, S], dt_a, kind="ExternalInput").ap()
    # host-pre-tiled: w1[e, ht, p(=d_in), dt, hi], w2[e, dt, p(=h_in), ht, di]
    w1 = nc.dram_tensor("w1", [EPC, NH, 128, ND, 128], dt_w, kind="ExternalInput").ap()
    w2 = nc.dram_tensor("w2", [EPC, ND, 128, NH, 128], dt_w2, kind="ExternalInput").ap()
    # host-pre-transposed biases: [p, e, col_tile]
    b1 = nc.dram_tensor("b1", [128, EPC, NH], f32, kind="ExternalInput").ap()
    b2 = nc.dram_tensor("b2", [128, EPC, ND], f32, kind="ExternalInput").ap()
    yT = nc.dram_tensor("yT", [D, S], dt_y, kind="ExternalOutput").ap()

    HNH = NH // 2  # half-layer column split
    HND = ND // 2
    # weight-pool lookahead: 4-byte weights are SBUF-tight
    WB = 3 if mybir.dt.size(dt_w) == 4 else 4

    HD = ND // 2  # d-tile half split for x tiles

    with tile.TileContext(nc) as tc, ExitStack() as ctx:
        wpool = ctx.enter_context(tc.tile_pool(name="w", bufs=4))
        xpool = ctx.enter_context(tc.tile_pool(name="x", bufs=2 * EPC))
        hpool = ctx.enter_context(tc.tile_pool(name="h", bufs=2 * NH))
        ypool = ctx.enter_context(tc.tile_pool(name="y", bufs=2))
        bpool = ctx.enter_context(tc.tile_pool(name="b", bufs=1))
        pp1 = ctx.enter_context(tc.tile_pool(name="ps1", bufs=3, space="PSUM"))
        pp2 = ctx.enter_context(tc.tile_pool(name="ps2", bufs=3, space="PSUM"))
        ppw = ctx.enter_context(tc.tile_pool(name="psw", bufs=1, space="PSUM"))

        def emit_x_half(e, half):
            C = caps[e]
            o0 = int(offs[e])
            xh = xpool.tile([128, HD * C], dt_a, tag=f"xh{half}", bufs=2)
            nc.sync.dma_start(
                out=xh[:].rearrange("p (dt t) -> p dt t", dt=HD),
                in_=xT[half * HD * 128:(half + 1) * HD * 128, o0:o0 + C]
                .rearrange("(dt p) t -> p dt t", p=128),
            )
            return xh

        # All input streaming rides ONE HWDGE ring (sync), triggered in
        # consumption order.  The early window is HBM-bandwidth-bound, so
        # a single FIFO ring guarantees the packets of the first matmul's
        # inputs aren't time-sliced against lower-priority transfers.
        # Slot 0's x halves interleave with its W1 eighth-chunks so the
        # very first accumulation chain unblocks at the earliest moment.
        # Biases ride the scalar ring (tiny); y writebacks also scalar.
        xts = {}
        b1_sb = bpool.tile([128, EPC * NH], f32, tag="b1")
        b2_sb = bpool.tile([128, EPC * ND], f32, tag="b2")
        nc.scalar.dma_start(
            out=b1_sb[:].rearrange("p (e ht) -> p e ht", e=EPC), in_=b1[:])
        nc.scalar.dma_start(
            out=b2_sb[:].rearrange("p (e dt) -> p e dt", e=EPC), in_=b2[:])

        # PE warm-up: dense dummy matmuls with no DMA dependency keep the
        # PE busy from the end of the NEFF preamble until the first real
        # matmul's inputs land, so HAM un-throttles as early as possible.
        if N_WARMUP_MM:
            wu = bpool.tile([128, WARMUP_COLS], mybir.dt.bfloat16, tag="wu")
            nc.vector.memset(wu[:], 0.0)
            wups = ppw.tile([128, WARMUP_COLS], f32, tag="psw")
            for i in range(N_WARMUP_MM):
                nc.tensor.matmul(wups[:], wu[:, :128], wu[:],
                                 start=(i == 0), stop=(i == N_WARMUP_MM - 1))

        gelu = mybir.ActivationFunctionType.Gelu
        for e in range(EPC):
            C = caps[e]
            o0 = int(offs[e])
            # weights: W1 in quarter column chunks interleaved with the x
            # halves, in consumption order, so each slot's layer-1 inputs
            # trickle in at matching granularity and never stall the PE.
            n_chunks = 4
            csz = NH // n_chunks
            w1h = []
            for half in range(n_chunks):
                if half == 0:
                    xea = emit_x_half(e, 0)
                wt = wpool.tile([128, csz * ND * 128], dt_w,
                                tag="w1c4", bufs=8)
                nc.sync.dma_start(
                    out=wt[:].rearrange("p (ht dt hi) -> p ht dt hi", ht=csz, dt=ND),
                    in_=w1[e, half * csz:(half + 1) * csz].rearrange(
                        "ht p dt hi -> p ht dt hi"),
                )
                w1h.append(wt)
                if half == 0:
                    xeb = emit_x_half(e, 1)

            def xs(dt_i, lo, hi, C=C, xea=xea, xeb=xeb):
                xh = xea if dt_i < HD else xeb
                return xh[:, (dt_i % HD) * C + lo: (dt_i % HD) * C + hi]
            w2h = []
            for half in range(2):
                wt = wpool.tile([128, HND * NH * 128], dt_w2, tag="w2c",
                                bufs=WB)
                nc.sync.dma_start(
                    out=wt[:].rearrange("p (dt ht di) -> p dt ht di", dt=HND, ht=NH),
                    in_=w2[e, half * HND:(half + 1) * HND].rearrange(
                        "dt p ht di -> p dt ht di"),
                )
                w2h.append(wt)

            ch0 = 0
            for CW in _chunks_for(C):
                hts = []
                for ht in range(NH):
                    wt = w1h[ht // csz]
                    hoff = (ht % csz) * ND * 128
                    ps = pp1.tile([128, CW], f32, tag="ps1")
                    for dt_i in range(ND):
                        nc.tensor.matmul(
                            ps[:],
                            wt[:, hoff + dt_i * 128: hoff + (dt_i + 1) * 128],
                            xs(dt_i, ch0, ch0 + CW),
                            start=(dt_i == 0),
                            stop=(dt_i == ND - 1),
                        )
                    hsb = hpool.tile([128, CW], dt_a, tag="ht")
                    nc.scalar.activation(
                        hsb[:], ps[:], gelu,
                        bias=b1_sb[:, e * NH + ht: e * NH + ht + 1],
                    )
                    hts.append(hsb)
                ysb = ypool.tile([128, ND * CW], dt_y, tag="yt")
                for dt_i in range(ND):
                    wt = w2h[dt_i // HND]
                    doff = (dt_i % HND) * NH * 128
                    ps2 = pp2.tile([128, CW], f32, tag="ps2")
                    for ht in range(NH):
                        nc.tensor.matmul(
                            ps2[:],
                            wt[:, doff + ht * 128: doff + (ht + 1) * 128],
                            hts[ht][:],
                            start=(ht == 0),
                            stop=(ht == NH - 1),
                        )
                    nc.vector.tensor_scalar_add(
                        ysb[:, dt_i * CW:(dt_i + 1) * CW], ps2[:],
                        b2_sb[:, e * ND + dt_i: e * ND + dt_i + 1],
                    )
                    # y quarter-writebacks: each covers 2 d-tiles and is
                    # issued as soon as its bias-adds are done, so the
                    # final DMA after the last matmul is small.
                    if dt_i % 2 == 1:
                        q0_, q1_ = (dt_i - 1) * 128, (dt_i + 1) * 128
                        nc.scalar.dma_start(
                            out=yT[q0_:q1_, o0 + ch0: o0 + ch0 + CW]
                            .rearrange("(dt p) t -> p dt t", p=128),
                            in_=ysb[:, (dt_i - 1) * CW:(dt_i + 1) * CW]
                            .rearrange("p (dt t) -> p dt t", dt=2),
                        )
                ch0 += CW
    nc.compile()
    return nc


def _get_nc(caps, dt_w, dt_w2, dt_a, dt_y):
    key = (caps, dt_w, dt_w2, dt_a, dt_y, HWDGE_ONLY, N_WARMUP_MM, WARMUP_COLS)
    if key not in _NC_CACHE:
        _NC_CACHE[key] = _build_nc(caps, dt_w, dt_w2, dt_a, dt_y)
    return _NC_CACHE[key]


_ML_NAMES = {"float8e3": "float8_e3m4", "float8e4": "float8_e4m3fn",
             "float8e5": "float8_e5m2"}
_FP8_MAX = {"float8e3": 15.5, "float8e4": 240.0, "float8e5": 57344.0}


def _np_dt(name):
    if name == "float32":
        return np.dtype(np.float32)
    import ml_dtypes
    return np.dtype(getattr(ml_dtypes, _ML_NAMES.get(name, name)))


def _route(xf, Wg):
    """Replicates the reference gate exactly in f32 numpy."""
    logits = xf @ Wg                                     # [T, E]
    m = logits.max(-1, keepdims=True)
    ex = np.exp(logits - m)
    scores = ex / ex.sum(-1, keepdims=True)
    idx = np.argsort(-scores, axis=1, kind="stable")[:, :TOP_K]  # [T, k]
    tw = np.take_along_axis(scores, idx, 1)
    m2 = tw.max(-1, keepdims=True)
    e2 = np.exp(tw - m2)
    w = (e2 / e2.sum(-1, keepdims=True)).astype(np.float32)
    return idx.astype(np.int64), w


def kernel(x, Wg, W1, b1, W2, b2):
    global LAST_EXEC_TIME_NS
    from concourse import bass_utils

    dt_w, dt_w2, dt_a, dt_y = DT_W, DT_W2, DT_A, DT_Y
    orig_shape = x.shape
    x = np.asarray(x, dtype=np.float32)
    Wg = np.asarray(Wg, dtype=np.float32)
    W1 = np.asarray(W1, dtype=np.float32)
    b1 = np.asarray(b1, dtype=np.float32)
    W2 = np.asarray(W2, dtype=np.float32)
    b2 = np.asarray(b2, dtype=np.float32)
    xf = np.ascontiguousarray(x.reshape(-1, D))
    T = xf.shape[0]

    idx, w = _route(xf, Wg)

    # ---- dispatch: sorted-group slot capacities, balanced expert->core map
    flat_e = idx.reshape(-1)                 # [k*T]
    flat_t = np.repeat(np.arange(T), TOP_K)
    order = np.argsort(flat_e, kind="stable")
    counts = np.bincount(flat_e, minlength=E)

    rank = np.argsort(-counts, kind="stable")      # experts sorted desc
    caps = tuple(max(int(counts[rank[8 * j]]), 16)
                 for j in range(EPC))              # slot capacity (max of group)
    S = sum(caps)
    offs = np.concatenate([[0], np.cumsum(caps)]).astype(int)

    # expert -> (core, slot): slot j holds ranks [8j, 8j+8)
    exp_core = np.zeros(E, np.int64)
    exp_slot = np.zeros(E, np.int64)
    for j in range(EPC):
        for c in range(N_CORES):
            e_ = rank[8 * j + c]
            exp_core[e_] = c
            exp_slot[e_] = j

    starts = np.zeros(E + 1, np.int64)
    starts[1:] = np.cumsum(counts)
    se = flat_e[order]                      # expert of each sorted assignment
    pos = np.arange(TOP_K * T) - starts[se]
    core = exp_core[se]
    col = offs[exp_slot[se]] + pos          # column in that core's xT
    tok = flat_t[order]

    gidx = np.zeros((N_CORES, S), np.int64)
    for c in range(N_CORES):
        msel = core == c
        gidx[c, col[msel]] = tok[msel]

    np_w = _np_dt(dt_w)
    np_w2 = _np_dt(dt_w2)
    np_a = _np_dt(dt_a)
    xf_a = xf.astype(np_a, copy=False)
    # fp8 W2: scale into the format's range; the scale is folded into b2
    # on the device side and divided back out in the host combine.
    if dt_w2 in _FP8_MAX:
        fmax = _FP8_MAX[dt_w2]
        s2 = float(fmax / max(np.abs(W2).max(), 1e-30))
        W2s = np.clip(W2 * s2, -fmax, fmax)
        b2s = b2 * s2
    else:
        s2 = 1.0
        W2s = W2
        b2s = b2
    # pre-tile weights: w1 -> [e, ht, p(d_in), dt, hi], w2 -> [e, dt, p(h_in), ht, di]
    W1t = np.ascontiguousarray(
        W1.reshape(E, ND, 128, NH, 128).transpose(0, 3, 2, 1, 4).astype(np_w, copy=False))
    W2t = np.ascontiguousarray(
        W2s.reshape(E, NH, 128, ND, 128).transpose(0, 3, 2, 1, 4).astype(np_w2, copy=False))
    # pre-transpose biases to [p, e, col_tile]
    b1t = np.ascontiguousarray(b1.reshape(E, NH, 128).transpose(2, 0, 1))
    b2t = np.ascontiguousarray(b2s.reshape(E, ND, 128).transpose(2, 0, 1))

    in_maps = []
    for c in range(N_CORES):
        es = [int(rank[8 * j + c]) for j in range(EPC)]   # this core's experts
        in_maps.append({
            "xT": np.ascontiguousarray(xf_a[gidx[c]].T),
            "w1": W1t[es],
            "w2": W2t[es],
            "b1": np.ascontiguousarray(b1t[:, es]),
            "b2": np.ascontiguousarray(b2t[:, es]),
        })

    nc = _get_nc(caps, dt_w, dt_w2, dt_a, dt_y)
    trace = os.environ.get("MOE_TRACE", "0") == "1"
    res = bass_utils.run_bass_kernel_spmd(
        nc, in_maps, core_ids=list(range(N_CORES)), trace=trace,
    )
    LAST_EXEC_TIME_NS = res.exec_time_ns

    # ---- combine: gather each (token, k) contribution, weight, and sum
    Ystack = np.stack([res.results[c]["yT"].astype(np.float32).T
                       for c in range(N_CORES)])
    contrib = Ystack[core, col]              # [k*T, D] (sorted order)
    inv = np.empty_like(order)
    inv[order] = np.arange(TOP_K * T)
    contrib = contrib[inv].reshape(T, TOP_K, D)
    wc = w * np.float32(1.0 / s2)          # fold fp8 W2 dequant into combine
    y = (contrib * wc[:, :, None]).sum(1).astype(np.float32)
    return y.reshape(orig_shape)


# revision 46
# speedup vs baseline: 1.0344x; 1.0344x over previous
"""MoE MLP (E=32 experts, top-2, D=H=1024) on 8 Trainium2 NeuronCores.

Strategy (expert parallel, per sharding hint):
  * Host computes the (tiny) gate: softmax(x @ Wg), top-2, renormalized
    weights, and dispatches tokens per expert into capacity-padded blocks,
    transposed to [D, tokens] (features on SBUF partitions, tokens on the
    matmul moving/free dimension). This is the sharding/all-to-all step.
  * Each of the 8 cores owns 4 experts (W1/W2/b1/b2 shards) and computes
    GELU(x W1 + b1) W2 + b2 for its experts' token blocks.
  * Host combines with the top-2 gate weights (scatter-add).

v2 changes vs the 90us baseline:
  * Variable per-slot capacities: experts are sorted by token count and
    grouped into 4 "slots" of 8; slot capacity = max count in the group
    (rounded to 16). All cores run the same program (SPMD) but total
    padded columns drop from 4*304=1216 to ~1088, cutting the matmul
    stream time proportionally (the kernel is PE-bound at ~0.42ns/col).
  * y written back in bf16 (halves writeback traffic; combine is f32 on
    host).
  * Warmup right-sized: dense 128-col dummy matmuls bridge the NEFF
    preamble to the first real matmul with no PE gap, so the HAM clock
    un-throttles (1.2 -> 2.4 GHz) as early as possible.
  * First slot's x and W1 issued first in small chunks for the earliest
    possible compute start.
"""

import os
import sys
import numpy as np

for _p in ("/root/.axon_site/_ro/trn_rl_repo", "/opt/trn_rl_repo"):
    if _p not in sys.path and os.path.isdir(_p):
        sys.path.append(_p)

E, D, H = 32, 1024, 1024
TOP_K = 2
N_CORES = 8
EPC = E // N_CORES  # experts (slots) per core
ND = D // 128       # d 128-tiles
NH = H // 128       # h 128-tiles

# weight dtype, activation dtype (must both be 16-bit or both 32-bit)
DT_W = os.environ.get("MOE_DT_W", "bfloat16")
DT_W2 = os.environ.get("MOE_DT_W2", "float8e3")   # E3M4 fp8 for W2
DT_A = os.environ.get("MOE_DT_A", "bfloat16")
DT_Y = os.environ.get("MOE_DT_Y", "bfloat16")
N_WARMUP_MM = int(os.environ.get("MOE_WARMUP", "31"))
WARMUP_COLS = int(os.environ.get("MOE_WARMUP_COLS", "256"))
HWDGE_ONLY = os.environ.get("MOE_HWDGE", "0") == "1"

LAST_EXEC_TIME_NS = None

_NC_CACHE = {}


def _chunks_for(cap):
    """Split a slot capacity into matmul chunks of <=512 columns."""
    n = -(-cap // 512)
    w = -(-cap // (n * 16)) * 16
    out = []
    rem = cap
    for _ in range(n):
        c = min(w, rem)
        out.append(c)
        rem -= c
    return tuple(out)


def _build_nc(caps, dt_w_name, dt_w2_name, dt_a_name, dt_y_name,
              hwdge=HWDGE_ONLY):
    import concourse.bass as bass  # noqa: F401
    import concourse.tile as tile
    from concourse import bacc, mybir
    from contextlib import ExitStack

    f32 = mybir.dt.float32
    dt_w = getattr(mybir.dt, dt_w_name)
    dt_w2 = getattr(mybir.dt, dt_w2_name)
    dt_a = getattr(mybir.dt, dt_a_name)
    dt_y = getattr(mybir.dt, dt_y_name)
    S = sum(caps)
    offs = np.concatenate([[0], np.cumsum(caps)]).astype(int)

    nc = bacc.Bacc(
        "TRN2",
        target_bir_lowering=False,
        debug=False,
        enable_asserts=False,
        num_devices=N_CORES,
    )
    xT = nc.dram_tensor("xT", [D, S], dt_a, kind="ExternalInput").ap()
    # host-pre-tiled: w1[e, ht, p(=d_in), dt, hi], w2[e, dt, p(=h_in), ht, di]
    w1 = nc.dram_tensor("w1", [EPC, NH, 128, ND, 128], dt_w, kind="ExternalInput").ap()
    w2 = nc.dram_tensor("w2", [EPC, ND, 128, NH, 128], dt_w2, kind="ExternalInput").ap()
    # host-pre-transposed biases: [p, e, col_tile]
    b1 = nc.dram_tensor("b1", [128, EPC, NH], f32, kind="ExternalInput").ap()
    b2 = nc.dram_tensor("b2", [128, EPC, ND], f32, kind="ExternalInput").ap()
    yT = nc.dram_tensor("yT", [D, S], dt_y, kind="ExternalOutput").ap()

    HNH = NH // 2  # half-layer column split
    HND = ND // 2
    # weight-pool lookahead: 4-byte weights are SBUF-tight
    WB = 3 if mybir.dt.size(dt_w) == 4 else 4

    HD = ND // 2  # d-tile half split for x tiles

    with tile.TileContext(nc) as tc, ExitStack() as ctx:
        wpool = ctx.enter_context(tc.tile_pool(name="w", bufs=4))
        xpool = ctx.enter_context(tc.tile_pool(name="x", bufs=2 * EPC))
        hpool = ctx.enter_context(tc.tile_pool(name="h", bufs=2 * NH))
        ypool = ctx.enter_context(tc.tile_pool(name="y", bufs=2))
        bpool = ctx.enter_context(tc.tile_pool(name="b", bufs=1))
        pp1 = ctx.enter_context(tc.tile_pool(name="ps1", bufs=3, space="PSUM"))
        pp2 = ctx.enter_context(tc.tile_pool(name="ps2", bufs=3, space="PSUM"))
        ppw = ctx.enter_context(tc.tile_pool(name="psw", bufs=1, space="PSUM"))

        def emit_x_half(e, half):
            C = caps[e]
            o0 = int(offs[e])
            xh = xpool.tile([128, HD * C], dt_a, tag=f"xh{half}", bufs=2)
            nc.sync.dma_start(
                out=xh[:].rearrange("p (dt t) -> p dt t", dt=HD),
                in_=xT[half * HD * 128:(half + 1) * HD * 128, o0:o0 + C]
                .rearrange("(dt p) t -> p dt t", p=128),
            )
            return xh

        # All input streaming rides ONE HWDGE ring (sync), triggered in
        # consumption order.  The early window is HBM-bandwidth-bound, so
        # a single FIFO ring guarantees the packets of the first matmul's
        # inputs aren't time-sliced against lower-priority transfers.
        # Slot 0's x halves interleave with its W1 eighth-chunks so the
        # very first accumulation chain unblocks at the earliest moment.
        # Biases ride the scalar ring (tiny); y writebacks also scalar.
        xts = {}
        b1_sb = bpool.tile([128, EPC * NH], f32, tag="b1")
        b2_sb = bpool.tile([128, EPC * ND], f32, tag="b2")
        nc.scalar.dma_start(
            out=b1_sb[:].rearrange("p (e ht) -> p e ht", e=EPC), in_=b1[:])
        nc.scalar.dma_start(
            out=b2_sb[:].rearrange("p (e dt) -> p e dt", e=EPC), in_=b2[:])

        # PE warm-up: dense dummy matmuls with no DMA dependency keep the
        # PE busy from the end of the NEFF preamble until the first real
        # matmul's inputs land, so HAM un-throttles as early as possible.
        if N_WARMUP_MM:
            wu = bpool.tile([128, WARMUP_COLS], mybir.dt.bfloat16, tag="wu")
            nc.vector.memset(wu[:], 0.0)
            wups = ppw.tile([128, WARMUP_COLS], f32, tag="psw")
            for i in range(N_WARMUP_MM):
                nc.tensor.matmul(wups[:], wu[:, :128], wu[:],
                                 start=(i == 0), stop=(i == N_WARMUP_MM - 1))

        gelu = mybir.ActivationFunctionType.Gelu
        for e in range(EPC):
            C = caps[e]
            o0 = int(offs[e])
            # weights: W1 in quarter column chunks interleaved with the x
            # halves, in consumption order, so each slot's layer-1 inputs
            # trickle in at matching granularity and never stall the PE.
            n_chunks = 4
            csz = NH // n_chunks
            w1h = []
            for half in range(n_chunks):
                if half == 0:
                    xea = emit_x_half(e, 0)
                wt = wpool.tile([128, csz * ND * 128], dt_w,
                                tag="w1c4", bufs=8)
                nc.sync.dma_start(
                    out=wt[:].rearrange("p (ht dt hi) -> p ht dt hi", ht=csz, dt=ND),
                    in_=w1[e, half * csz:(half + 1) * csz].rearrange(
                        "ht p dt hi -> p ht dt hi"),
                )
                w1h.append(wt)
                if half == 0:
                    xeb = emit_x_half(e, 1)

            def xs(dt_i, lo, hi, C=C, xea=xea, xeb=xeb):
                xh = xea if dt_i < HD else xeb
                return xh[:, (dt_i % HD) * C + lo: (dt_i % HD) * C + hi]
            w2h = []
            for half in range(2):
                wt = wpool.tile([128, HND * NH * 128], dt_w2, tag="w2c",
                                bufs=WB)
                nc.sync.dma_start(
                    out=wt[:].rearrange("p (dt ht di) -> p dt ht di", dt=HND, ht=NH),
                    in_=w2[e, half * HND:(half + 1) * HND].rearrange(
                        "dt p ht di -> p dt ht di"),
                )
                w2h.append(wt)

            ch0 = 0
            for CW in _chunks_for(C):
                hts = []
                for ht in range(NH):
                    wt = w1h[ht // csz]
                    hoff = (ht % csz) * ND * 128
                    ps = pp1.tile([128, CW], f32, tag="ps1")
                    for dt_i in range(ND):
                        nc.tensor.matmul(
                            ps[:],
                            wt[:, hoff + dt_i * 128: hoff + (dt_i + 1) * 128],
                            xs(dt_i, ch0, ch0 + CW),
                            start=(dt_i == 0),
                            stop=(dt_i == ND - 1),
                        )
                    hsb = hpool.tile([128, CW], dt_a, tag="ht")
                    nc.scalar.activation(
                        hsb[:], ps[:], gelu,
                        bias=b1_sb[:, e * NH + ht: e * NH + ht + 1],
                    )
                    hts.append(hsb)
                ysb = ypool.tile([128, ND * CW], dt_y, tag="yt")
                for dt_i in range(ND):
                    wt = w2h[dt_i // HND]
                    doff = (dt_i % HND) * NH * 128
                    ps2 = pp2.tile([128, CW], f32, tag="ps2")
                    for ht in range(NH):
                        nc.tensor.matmul(
                            ps2[:],
                            wt[:, doff + ht * 128: doff + (ht + 1) * 128],
                            hts[ht][:],
                            start=(ht == 0),
                            stop=(ht == NH - 1),
                        )
                    nc.vector.tensor_scalar_add(
                        ysb[:, dt_i * CW:(dt_i + 1) * CW], ps2[:],
                        b2_sb[:, e * ND + dt_i: e * ND + dt_i + 1],
                    )
                    # y quarter-writebacks: each covers 2 d-tiles and is
                    # issued as soon as its bias-adds are done, so the
                    # final DMA after the last matmul is small.
                    if dt_i % 2 == 1:
                        q0_, q1_ = (dt_i - 1) * 128, (dt_i + 1) * 128
                        nc.scalar.dma_start(
                            out=yT[q0_:q1_, o0 + ch0: o0 + ch0 + CW]
                            .rearrange("(dt p) t -> p dt t", p=128),
                            in_=ysb[:, (dt_i - 1) * CW:(dt_i + 1) * CW]
                            .rearrange("p (dt t) -> p dt t", dt=2),
                        )
                ch0 += CW
    nc.compile()
    return nc


def _get_nc(caps, dt_w, dt_w2, dt_a, dt_y):
    key = (caps, dt_w, dt_w2, dt_a, dt_y, HWDGE_ONLY, N_WARMUP_MM, WARMUP_COLS)
    if key not in _NC_CACHE:
        _NC_CACHE[key] = _build_nc(caps, dt_w, dt_w2, dt_a, dt_y)
    return _NC_CACHE[key]


_ML_NAMES = {"float8e3": "float8_e3m4", "float8e4": "float8_e4m3fn",
             "float8e5": "float8_e5m2"}
_FP8_MAX = {"float8e3": 15.5, "float8e4": 240.0, "float8e5": 57344.0}


def _np_dt(name):
    if name == "float32":
        return np.dtype(np.float32)
    import ml_dtypes
    return np.dtype(getattr(ml_dtypes, _ML_NAMES.get(name, name)))


def _route(xf, Wg):
    """Replicates the reference gate exactly in f32 numpy."""
    logits = xf @ Wg                                     # [T, E]
    m = logits.max(-1, keepdims=True)
    ex = np.exp(logits - m)
    scores = ex / ex.sum(-1, keepdims=True)
    idx = np.argsort(-scores, axis=1, kind="stable")[:, :TOP_K]  # [T, k]
    tw = np.take_along_axis(scores, idx, 1)
    m2 = tw.max(-1, keepdims=True)
    e2 = np.exp(tw - m2)
    w = (e2 / e2.sum(-1, keepdims=True)).astype(np.float32)
    return idx.astype(np.int64), w


def kernel(x, Wg, W1, b1, W2, b2):
    global LAST_EXEC_TIME_NS
    from concourse import bass_utils

    dt_w, dt_w2, dt_a, dt_y = DT_W, DT_W2, DT_A, DT_Y
    orig_shape = x.shape
    x = np.asarray(x, dtype=np.float32)
    Wg = np.asarray(Wg, dtype=np.float32)
    W1 = np.asarray(W1, dtype=np.float32)
    b1 = np.asarray(b1, dtype=np.float32)
    W2 = np.asarray(W2, dtype=np.float32)
    b2 = np.asarray(b2, dtype=np.float32)
    xf = np.ascontiguousarray(x.reshape(-1, D))
    T = xf.shape[0]

    idx, w = _route(xf, Wg)

    # ---- dispatch: sorted-group slot capacities, balanced expert->core map
    flat_e = idx.reshape(-1)                 # [k*T]
    flat_t = np.repeat(np.arange(T), TOP_K)
    order = np.argsort(flat_e, kind="stable")
    counts = np.bincount(flat_e, minlength=E)

    rank = np.argsort(-counts, kind="stable")      # experts sorted desc
    caps = tuple(max(int(counts[rank[8 * j]]), 16)
                 for j in range(EPC))              # slot capacity (max of group)
    S = sum(caps)
    offs = np.concatenate([[0], np.cumsum(caps)]).astype(int)

    # expert -> (core, slot): slot j holds ranks [8j, 8j+8)
    exp_core = np.zeros(E, np.int64)
    exp_slot = np.zeros(E, np.int64)
    for j in range(EPC):
        for c in range(N_CORES):
            e_ = rank[8 * j + c]
            exp_core[e_] = c
            exp_slot[e_] = j

    starts = np.zeros(E + 1, np.int64)
    starts[1:] = np.cumsum(counts)
    se = flat_e[order]                      # expert of each sorted assignment
    pos = np.arange(TOP_K * T) - starts[se]
    core = exp_core[se]
    col = offs[exp_slot[se]] + pos          # column in that core's xT
    tok = flat_t[order]

    gidx = np.zeros((N_CORES, S), np.int64)
    for c in range(N_CORES):
        msel = core == c
        gidx[c, col[msel]] = tok[msel]

    np_w = _np_dt(dt_w)
    np_w2 = _np_dt(dt_w2)
    np_a = _np_dt(dt_a)
    xf_a = xf.astype(np_a, copy=False)
    # fp8 W2: scale into the format's range; the scale is folded into b2
    # on the device side and divided back out in the host combine.
    if dt_w2 in _FP8_MAX:
        fmax = _FP8_MAX[dt_w2]
        s2 = float(fmax / max(np.abs(W2).max(), 1e-30))
        W2s = np.clip(W2 * s2, -fmax, fmax)
        b2s = b2 * s2
    else:
        s2 = 1.0
        W2s = W2
        b2s = b2
    # pre-tile weights: w1 -> [e, ht, p(d_in), dt, hi], w2 -> [e, dt, p(h_in), ht, di]
    W1t = np.ascontiguousarray(
        W1.reshape(E, ND, 128, NH, 128).transpose(0, 3, 2, 1, 4).astype(np_w, copy=False))
    W2t = np.ascontiguousarray(
        W2s.reshape(E, NH, 128, ND, 128).transpose(0, 3, 2, 1, 4).astype(np_w2, copy=False))
    # pre-transpose biases to [p, e, col_tile]
    b1t = np.ascontiguousarray(b1.reshape(E, NH, 128).transpose(2, 0, 1))
    b2t = np.ascontiguousarray(b2s.reshape(E, ND, 128).transpose(2, 0, 1))

    in_maps = []
    for c in range(N_CORES):
        es = [int(rank[8 * j + c]) for j in range(EPC)]   # this core's experts
        in_maps.append({
            "xT": np.ascontiguousarray(xf_a[gidx[c]].T),
            "w1": W1t[es],
            "w2": W2t[es],
            "b1": np.ascontiguousarray(b1t[:, es]),
            "b2": np.ascontiguousarray(b2t[:, es]),
        })

    nc = _get_nc(caps, dt_w, dt_w2, dt_a, dt_y)
    trace = os.environ.get("MOE_TRACE", "0") == "1"
    res = bass_utils.run_bass_kernel_spmd(
        nc, in_maps, core_ids=list(range(N_CORES)), trace=trace,
    )
    LAST_EXEC_TIME_NS = res.exec_time_ns

    # ---- combine: gather each (token, k) contribution, weight, and sum
    Ystack = np.stack([res.results[c]["yT"].astype(np.float32).T
                       for c in range(N_CORES)])
    contrib = Ystack[core, col]              # [k*T, D] (sorted order)
    inv = np.empty_like(order)
    inv[order] = np.arange(TOP_K * T)
    contrib = contrib[inv].reshape(T, TOP_K, D)
    wc = w * np.float32(1.0 / s2)          # fold fp8 W2 dequant into combine
    y = (contrib * wc[:, :, None]).sum(1).astype(np.float32)
    return y.reshape(orig_shape)
